# revision 1
# baseline (speedup 1.0000x reference)
"""Trainium2 Bass kernel for nn_Block_15118284882621 (dense transformer block).

Sharding (8 NeuronCores, single chip):
  - qkv + MLP: token-parallel (1024 tokens/core, channel-major layout)
  - attention: head-pair-parallel (2 heads/core, all 8192 tokens)
  - connected by 4 AllToAlls (q, k, v, ctx: 4.2 MB each per rank)

All matmuls in float32r (fp32 storage, ~13-bit-mantissa PE mode, full rate).
Everything on device is channel-major (x^T); host transposes in/out.
Softmax: no max-subtraction (scores are O(1) for this problem); denominator
comes from a ones-column in the attn@v matmul; mask folded multiplicatively
via exp(rpb) (host-precomputed, bf16) and a per-k mask vector.
"""

import numpy as np
import ml_dtypes

import concourse.bass as bass
from concourse import bacc
import concourse.mybir as mybir
import concourse.tile as tile
from concourse.bass_utils import run_bass_kernel_spmd
from concourse.masks import make_identity

B, N, C, H, MT = 4, 2048, 1024, 16, 40
HD = C // H            # 64
HID = 4 * C            # 4096
SCALE = HD ** -0.5
EPS = 1e-5
NCORE = 8
TOK = B * N // NCORE   # 1024 tokens per core
HP = H // NCORE        # 2 heads per core
P = 128
F32 = mybir.dt.float32
F32R = mybir.dt.float32r
BF16 = mybir.dt.bfloat16
AF = mybir.ActivationFunctionType
ALU = mybir.AluOpType
RG = [[0, 1, 2, 3, 4, 5, 6, 7]]

CT = C // P            # 8 channel tiles
KT = N // P            # 16 k tiles per batch
QCH = 512              # matmul moving-chunk width
HTC = 16               # hidden tiles per MLP chunk (chunk = 2048)


PHASE_MARKS = []


def build_kernel(debug=False, apply_mask=True, skip_cc=False, skip_attn=False, skip_mlp=False):
    nc = bacc.Bacc(None, target_bir_lowering=False)
    PHASE_MARKS.clear()

    def _mark(name):
        PHASE_MARKS.append((name, len(nc.inst_map)))

    # ---------------- external I/O (per core) ----------------
    xT_d = nc.dram_tensor("xT", [C, TOK], F32R, kind="ExternalInput")
    wqkvT_d = nc.dram_tensor("wqkvT", [C, 3 * C], F32R, kind="ExternalInput")
    qkvb_d = nc.dram_tensor("qkvb", [P, 3 * CT], F32, kind="ExternalInput")
    erpbT_d = nc.dram_tensor("erpbT", [HP, N, N], BF16, kind="ExternalInput")
    maskm_d = nc.dram_tensor("maskm", [P, B * KT], F32R, kind="ExternalInput")
    wprojT_d = nc.dram_tensor("wprojT", [C, C], F32R, kind="ExternalInput")
    projbg_d = nc.dram_tensor("projbg", [P, CT], F32, kind="ExternalInput")
    g1_d = nc.dram_tensor("g1", [P, CT], F32, kind="ExternalInput")
    n1w_d = nc.dram_tensor("n1w", [P, CT], F32, kind="ExternalInput")
    n1b_d = nc.dram_tensor("n1b", [P, CT], F32, kind="ExternalInput")
    n2iw_d = nc.dram_tensor("n2iw", [P, CT], F32, kind="ExternalInput")
    n2ib_d = nc.dram_tensor("n2ib", [P, CT], F32, kind="ExternalInput")
    n2tw_d = nc.dram_tensor("n2tw", [P, CT], F32, kind="ExternalInput")
    n2tb_d = nc.dram_tensor("n2tb", [P, CT], F32, kind="ExternalInput")
    wfc1iT_d = nc.dram_tensor("wfc1iT", [C, HID], BF16, kind="ExternalInput")
    bfc1i_d = nc.dram_tensor("bfc1i", [P, HID // P], F32, kind="ExternalInput")
    wfc2iT_d = nc.dram_tensor("wfc2iT", [HID, C], BF16, kind="ExternalInput")
    g2b2i_d = nc.dram_tensor("g2b2i", [P, CT], F32, kind="ExternalInput")
    wfc1tT_d = nc.dram_tensor("wfc1tT", [C, HID], BF16, kind="ExternalInput")
    bfc1t_d = nc.dram_tensor("bfc1t", [P, HID // P], F32, kind="ExternalInput")
    wfc2tT_d = nc.dram_tensor("wfc2tT", [HID, C], BF16, kind="ExternalInput")
    g2b2t_d = nc.dram_tensor("g2b2t", [P, CT], F32, kind="ExternalInput")
    g2_d = nc.dram_tensor("g2", [P, CT], F32, kind="ExternalInput")
    sel_d = nc.dram_tensor("sel", [P, 1], F32, kind="ExternalInput")

    outT_d = nc.dram_tensor("outT", [C, TOK], F32, kind="ExternalOutput")
    if debug:
        dbg_xn_d = nc.dram_tensor("dbg_xn", [C, TOK], F32, kind="ExternalOutput")
        dbg_ctx_d = nc.dram_tensor("dbg_ctx", [P, B * N], F32, kind="ExternalOutput")
        dbg_x1_d = nc.dram_tensor("dbg_x1", [C, TOK], F32, kind="ExternalOutput")
        dbg_mlp_d = nc.dram_tensor("dbg_mlp", [C, TOK], F32, kind="ExternalOutput")

    with tile.TileContext(nc) as tc:
        with tc.tile_pool(name="const", bufs=1) as const, \
             tc.tile_pool(name="dram", bufs=1, space="DRAM") as dram:

            ident_f = const.tile([P, P], F32, name="ident_f", tag="ident_f")
            make_identity(nc, ident_f)
            ident = const.tile([P, P], F32R, name="ident", tag="ident")
            nc.vector.tensor_copy(out=ident[:], in_=ident_f[:])
            ones_f = const.tile([P, 1], F32, name="ones_f", tag="ones_f")
            nc.vector.memset(ones_f[:], 1.0)
            ones_v = const.tile([P, 1], F32R, name="ones_v", tag="ones_v")
            nc.vector.tensor_copy(out=ones_v[:], in_=ones_f[:])

            def load_const(name, dten, cols):
                t = const.tile([P, cols], F32, name=name, tag=name)
                nc.sync.dma_start(out=t[:], in_=dten.ap())
                return t

            qkvb_sb = load_const("qkvb_sb", qkvb_d, 3 * CT)
            maskm_sb = const.tile([P, B * KT], F32R, name="maskm_sb", tag="maskm_sb")
            nc.sync.dma_start(out=maskm_sb[:], in_=maskm_d.ap())
            projbg_sb = load_const("projbg_sb", projbg_d, CT)
            g1_sb = load_const("g1_sb", g1_d, CT)
            n1w_sb = load_const("n1w_sb", n1w_d, CT)
            n1b_sb = load_const("n1b_sb", n1b_d, CT)
            n2iw_sb = load_const("n2iw_sb", n2iw_d, CT)
            n2ib_sb = load_const("n2ib_sb", n2ib_d, CT)
            n2tw_sb = load_const("n2tw_sb", n2tw_d, CT)
            n2tb_sb = load_const("n2tb_sb", n2tb_d, CT)
            bfc1i_sb = load_const("bfc1i_sb", bfc1i_d, HID // P)
            g2b2i_sb = load_const("g2b2i_sb", g2b2i_d, CT)
            bfc1t_sb = load_const("bfc1t_sb", bfc1t_d, HID // P)
            g2b2t_sb = load_const("g2b2t_sb", g2b2t_d, CT)
            g2_sb = load_const("g2_sb", g2_d, CT)
            sel_sb = load_const("sel_sb", sel_d, 1)

            # DRAM bounce buffers for collectives
            def dram_buf(name, shape):
                return dram.tile(shape, F32, name=name, tag=name)

            inqh = [dram_buf(f"inq{h}", [NCORE, HD, TOK]) for h in range(HP)]
            inkh = [dram_buf(f"ink{h}", [NCORE, HD, TOK]) for h in range(HP)]
            invh = [dram_buf(f"inv{h}", [NCORE, TOK, HD]) for h in range(HP)]
            outqh = [dram_buf(f"outq{h}", [NCORE, HD, TOK]) for h in range(HP)]
            outkh = [dram_buf(f"outk{h}", [NCORE, HD, TOK]) for h in range(HP)]
            outvh = [dram_buf(f"outv{h}", [NCORE, TOK, HD]) for h in range(HP)]
            in2 = dram_buf("in2", [NCORE, P, TOK])
            out2 = dram_buf("out2", [NCORE, P, TOK])

            # lifetime-scoped static pools (entered/exited manually)
            cm_xt = tc.tile_pool(name="life_xt", bufs=1)
            pool_xt = cm_xt.__enter__()
            cm_xn = tc.tile_pool(name="life_xn", bufs=1, side="right")
            pool_xn = cm_xn.__enter__()
            xt = [pool_xt.tile([P, TOK], F32R, name=f"xt{i}", tag=f"xt{i}") for i in range(CT)]
            xn = [pool_xn.tile([P, TOK], F32R, name=f"xn{i}", tag=f"xn{i}") for i in range(CT)]

            # ---------- channel-major layernorm over the channel axis ----------
            def layernorm(src_tiles, cols, w_sb, wi, b_sb, out_tiles, scratch, psump):
                sums = psump.tile([1, cols], F32, name="ln_sums", tag="ln_sums")
                sumsq = psump.tile([1, cols], F32, name="ln_sumsq", tag="ln_sumsq")
                nchunk = (cols + QCH - 1) // QCH
                for i in range(CT):
                    sq = scratch.tile([P, cols], F32R, name=f"ln_sq{i}", tag="ln_sq")
                    nc.vector.tensor_mul(sq[:],
                                         src_tiles[i][:, 0:cols].bitcast(F32),
                                         src_tiles[i][:, 0:cols].bitcast(F32))
                    for cc in range(nchunk):
                        cs = slice(cc * QCH, min((cc + 1) * QCH, cols))
                        nc.tensor.matmul(sums[:, cs], ones_v[:], src_tiles[i][:, cs],
                                         start=(i == 0), stop=(i == CT - 1))
                        nc.tensor.matmul(sumsq[:, cs], ones_v[:], sq[:, cs],
                                         start=(i == 0), stop=(i == CT - 1))
                mu = scratch.tile([1, cols], F32, name="ln_mu", tag="ln_mu")
                nc.vector.tensor_scalar_mul(mu[:], sums[:], 1.0 / C)
                msq = scratch.tile([1, cols], F32, name="ln_msq", tag="ln_msq")
                nc.vector.tensor_scalar_mul(msq[:], sumsq[:], 1.0 / C)
                var = scratch.tile([1, cols], F32, name="ln_var", tag="ln_var")
                nc.vector.scalar_tensor_tensor(var[:], mu[:], -1.0, mu[:],
                                               ALU.mult, ALU.mult)
                nc.vector.tensor_add(var[:], var[:], msq[:])
                nc.vector.tensor_scalar_add(var[:], var[:], EPS)
                sd = scratch.tile([1, cols], F32, name="ln_sd", tag="ln_sd")
                nc.scalar.activation(sd[:], var[:], AF.Sqrt)
                rstd = scratch.tile([1, cols], F32, name="ln_rstd", tag="ln_rstd")
                nc.vector.reciprocal(rstd[:], sd[:])
                mub = scratch.tile([P, cols], F32, name="ln_mub", tag="ln_mub")
                nc.gpsimd.partition_broadcast(mub[:], mu[:], channels=P)
                rstdb = scratch.tile([P, cols], F32, name="ln_rstdb", tag="ln_rstdb")
                nc.gpsimd.partition_broadcast(rstdb[:], rstd[:], channels=P)
                for i in range(CT):
                    t1 = scratch.tile([P, cols], F32, name=f"ln_t1_{i}", tag="ln_t1")
                    nc.vector.tensor_sub(t1[:], src_tiles[i][:, 0:cols].bitcast(F32), mub[:])
                    t2 = scratch.tile([P, cols], F32, name=f"ln_t2_{i}", tag="ln_t2")
                    nc.vector.tensor_mul(t2[:], t1[:], rstdb[:])
                    nc.vector.scalar_tensor_tensor(
                        out_tiles[i][:, 0:cols], t2[:], w_sb[:, wi:wi + 1],
                        b_sb[:, wi:wi + 1].broadcast_to([P, cols]), ALU.mult, ALU.add)

            def wrow(wpool, dten, prow, c0, c1, tag, dtype=F32R):
                """Load one [P, c1-c0] row-band of a weight matrix as a big tile."""
                wt = wpool.tile([P, c1 - c0], dtype, name=f"w_{tag}_{prow}_{c0}", tag=tag)
                nc.sync.dma_start(out=wt[:], in_=dten.ap()[prow * P:(prow + 1) * P, c0:c1])
                return wt

            # ================= P0: load x, LN1 =================
            with tc.tile_pool(name="p0", bufs=2) as scratch, \
                 tc.tile_pool(name="p0psum", bufs=1, space="PSUM") as psump:
                for i in range(CT):
                    nc.sync.dma_start(out=xt[i][:], in_=xT_d.ap()[i * P:(i + 1) * P, :])
                layernorm(xt, TOK, n1w_sb, 0, n1b_sb, xn, scratch, psump)
                if debug:
                    for i in range(CT):
                        nc.sync.dma_start(out=dbg_xn_d.ap()[i * P:(i + 1) * P, :],
                                          in_=xn[i][:].bitcast(F32))

            _mark("p0_ln1")
            # ================= P1: qkv + v transpose -> A2A inputs =================
            with tc.tile_pool(name="p1w", bufs=1) as wpool, \
                 tc.tile_pool(name="p1s", bufs=3) as spool, \
                 tc.tile_pool(name="p1v", bufs=2) as vpool, \
                 tc.tile_pool(name="p1psum", bufs=3, space="PSUM") as pp, \
                 tc.tile_pool(name="p1psumT", bufs=2, space="PSUM") as ppt:
                for tg in range(3):                      # t-groups of 8 ch-tiles
                    wq = [wrow(wpool, wqkvT_d, k, tg * C, (tg + 1) * C, f"wq{k}")
                          for k in range(CT)]
                    for tt in range(CT):
                        t = tg * CT + tt
                        hp_idx, kind = t // 3, t % 3     # 0=q,1=k,2=v
                        pq = pp.tile([P, TOK], F32, name=f"pq{t}", tag="pq")
                        for k in range(CT):
                            for cc in range(TOK // QCH):
                                cs = slice(cc * QCH, (cc + 1) * QCH)
                                nc.tensor.matmul(pq[:, cs], wq[k][:, tt * P:(tt + 1) * P],
                                                 xn[k][:, cs], start=(k == 0),
                                                 stop=(k == CT - 1))
                        qv = spool.tile([P, TOK], F32R, name=f"qv{t}", tag="qv")
                        nc.scalar.activation(qv[:], pq[:], AF.Identity,
                                             bias=qkvb_sb[:, t:t + 1])
                        if kind == 0:
                            for h in range(HP):
                                nc.sync.dma_start(out=inqh[h][hp_idx],
                                                  in_=qv[h * HD:(h + 1) * HD, :].bitcast(F32))
                        elif kind == 1:
                            for h in range(HP):
                                nc.sync.dma_start(out=inkh[h][hp_idx],
                                                  in_=qv[h * HD:(h + 1) * HD, :].bitcast(F32))
                        else:
                            vtb = [vpool.tile([P, TOK // P, HD], F32R,
                                              name=f"vtb{t}_{h}", tag=f"vtb{h}")
                                   for h in range(HP)]
                            for c8 in range(TOK // P):
                                tp = ppt.tile([P, P], F32R, name=f"tp{t}_{c8}", tag="tp")
                                nc.tensor.transpose(tp[:], qv[:, c8 * P:(c8 + 1) * P],
                                                    ident[:])
                                for h in range(HP):
                                    nc.vector.tensor_copy(
                                        out=vtb[h][:, c8, :],
                                        in_=tp[:, h * HD:(h + 1) * HD])
                            for h in range(HP):
                                nc.sync.dma_start(
                                    out=invh[h][hp_idx].rearrange(
                                        "(c p) d -> p c d", p=P).bitcast(F32R),
                                    in_=vtb[h][:])
            cm_xn.__exit__(None, None, None)  # xn dead after qkv
            cm_xt.__exit__(None, None, None)  # x reloaded from DRAM at P5

            _mark("p1_qkv")
            # ================= P2: qkv A2As =================
            if skip_cc:
                outqh, outkh, outvh = inqh, inkh, invh
            else:
                for h in range(HP):
                    nc.gpsimd.collective_compute("AllToAll", ALU.bypass, ins=[inqh[h][:]],
                                                 outs=[outqh[h][:]], replica_groups=RG)
                    nc.gpsimd.collective_compute("AllToAll", ALU.bypass, ins=[inkh[h][:]],
                                                 outs=[outkh[h][:]], replica_groups=RG)
                    nc.gpsimd.collective_compute("AllToAll", ALU.bypass, ins=[invh[h][:]],
                                                 outs=[outvh[h][:]], replica_groups=RG)

            _mark("p2_a2a")
            # ================= P3: attention =================
            cm_ctx = tc.tile_pool(name="life_ctx", bufs=1, side="right")
            pool_ctx = cm_ctx.__enter__()
            ctx_sb = pool_ctx.tile([P, B * N], F32R, name="ctx_sb", tag="ctx_sb")
            if skip_attn:
                nc.vector.tensor_copy(out=ctx_sb[:],
                                      in_=ones_f[:, :].broadcast_to([P, B * N]))
            with tc.tile_pool(name="p3qk", bufs=1) as qkpool, \
                 tc.tile_pool(name="p3va", bufs=1) as vapool, \
                 tc.tile_pool(name="p3s", bufs=2) as spool, \
                 tc.tile_pool(name="p3er", bufs=3) as erpool, \
                 tc.tile_pool(name="p3ps", bufs=1, space="PSUM") as psS, \
                 tc.tile_pool(name="p3pc", bufs=1, space="PSUM") as psC:
                for h in range(0 if skip_attn else HP):
                    # paired q/k tiles: rows 0:64 = batch 2j, 64:128 = batch 2j+1
                    qh2, kh2 = [], []
                    for j in range(B // 2):
                        qt = qkpool.tile([P, N], F32R, name=f"qh{h}_{j}", tag=f"qh{j}")
                        kt_ = qkpool.tile([P, N], F32R, name=f"kh{h}_{j}", tag=f"kh{j}")
                        for bb in range(2):
                            for half in range(2):
                                srcc = 2 * (2 * j + bb) + half
                                dst = slice(half * TOK, (half + 1) * TOK)
                                nc.sync.dma_start(out=qt[bb * HD:(bb + 1) * HD, dst],
                                                  in_=outqh[h][srcc].bitcast(F32R))
                                nc.sync.dma_start(out=kt_[bb * HD:(bb + 1) * HD, dst],
                                                  in_=outkh[h][srcc].bitcast(F32R))
                        qh2.append(qt)
                        kh2.append(kt_)

                    def qk_ap(lst, b, cols):
                        t = lst[b // 2]
                        r = (b % 2) * HD
                        return t[r:r + HD, cols]

                    # v tiles: [128 tok-part, 8 k-chunks, 65] (col 64 = ones)
                    vbig = {}
                    for b in range(B):
                        for half in range(2):
                            vt_ = vapool.tile([P, KT // 2, HD + 1], F32R,
                                              name=f"vb{h}_{b}_{half}", tag=f"vb{b}_{half}")
                            nc.sync.dma_start(
                                out=vt_[:, :, 0:HD],
                                in_=outvh[h][2 * b + half].rearrange(
                                    "(c p) d -> p c d", p=P).bitcast(F32R))
                            nc.vector.tensor_copy(
                                out=vt_[:, :, HD:HD + 1],
                                in_=ones_f[:, None, :].broadcast_to([P, KT // 2, 1]))
                            if apply_mask:
                                for c8 in range(KT // 2):
                                    mi = b * KT + half * (KT // 2) + c8
                                    nc.vector.tensor_scalar_mul(
                                        vt_[:, c8, :], vt_[:, c8, :],
                                        maskm_sb[:, mi:mi + 1].bitcast(F32))
                            vbig[(b, half)] = vt_

                    for qq in range(N // QCH):
                        qs = slice(qq * QCH, (qq + 1) * QCH)
                        ctxp = [psC.tile([HD + 1, QCH], F32,
                                         name=f"ctxp{h}_{qq}_{b}", tag=f"ctxp{b}")
                                for b in range(B)]
                        for kt in range(KT):
                            er = erpool.tile([P, QCH], BF16, name=f"er{h}_{qq}_{kt}", tag="er")
                            nc.sync.dma_start(out=er[:],
                                              in_=erpbT_d.ap()[h, kt * P:(kt + 1) * P, qs])
                            sp = psS.tile([P, B, QCH], F32,
                                          name=f"sp{h}_{qq}_{kt}", tag="sp")
                            for b in range(B):
                                nc.tensor.matmul(sp[:, b, :],
                                                 qk_ap(kh2, b, slice(kt * P, (kt + 1) * P)),
                                                 qk_ap(qh2, b, qs), start=True, stop=True,
                                                 skip_group_check=True)
                            tmp = spool.tile([P, B, QCH], F32,
                                             name=f"tm{h}_{qq}_{kt}", tag="tmp")
                            nc.scalar.activation(tmp[:], sp[:], AF.Exp)
                            pkt = spool.tile([P, B, QCH], F32R,
                                             name=f"pk{h}_{qq}_{kt}", tag="pkt")
                            erb = er[:, None, :].broadcast_to([P, B, QCH])
                            if kt % 3 == 2:
                                nc.gpsimd.tensor_mul(pkt[:], tmp[:], erb)
                            else:
                                nc.vector.tensor_mul(pkt[:], tmp[:], erb)
                            for b in range(B):
                                nc.tensor.matmul(
                                    ctxp[b][:], vbig[(b, kt // (KT // 2))][:, kt % (KT // 2), :],
                                    pkt[:, b, :], start=(kt == 0), stop=(kt == KT - 1))
                        for b in range(B):
                            rec = spool.tile([1, QCH], F32, name=f"rec{h}_{qq}_{b}", tag="rec")
                            nc.vector.reciprocal(rec[:], ctxp[b][HD:HD + 1, :])
                            recb = spool.tile([HD, QCH], F32,
                                              name=f"recb{h}_{qq}_{b}", tag="recb")
                            nc.gpsimd.partition_broadcast(recb[:], rec[:], channels=HD)
                            nc.vector.tensor_mul(
                                ctx_sb[h * HD:(h + 1) * HD,
                                       b * N + qq * QCH: b * N + (qq + 1) * QCH],
                                ctxp[b][0:HD, :], recb[:])

            _mark("p3_attn")
            # ================= P4: ctx A2A =================
            for j in range(NCORE):
                nc.sync.dma_start(out=in2[j],
                                  in_=ctx_sb[:, j * TOK:(j + 1) * TOK].bitcast(F32))
            if skip_cc:
                out2 = in2
            else:
                nc.gpsimd.collective_compute("AllToAll", ALU.bypass, ins=[in2[:]],
                                             outs=[out2[:]], replica_groups=RG)
            if debug:
                nc.sync.dma_start(out=dbg_ctx_d.ap(), in_=ctx_sb[:].bitcast(F32))
            cm_ctx.__exit__(None, None, None)

            _mark("p4_a2a")
            # ================= P5: proj + residual =================
            cm_x1 = tc.tile_pool(name="life_x1", bufs=1, side="right")
            pool_x1 = cm_x1.__enter__()
            x1 = [pool_x1.tile([P, TOK], F32R, name=f"x1_{i}", tag=f"x1_{i}")
                  for i in range(CT)]
            with tc.tile_pool(name="p5c", bufs=1) as cpool, \
                 tc.tile_pool(name="p5", bufs=2) as spool, \
                 tc.tile_pool(name="p5w", bufs=1) as wpool, \
                 tc.tile_pool(name="p5psum", bufs=3, space="PSUM") as pp:
                ctxf = []
                xres = []
                for i in range(CT):
                    cf = cpool.tile([P, TOK], F32R, name=f"ctxf{i}", tag=f"ctxf{i}")
                    nc.sync.dma_start(out=cf[:], in_=out2[i].bitcast(F32R))
                    ctxf.append(cf)
                    xr = cpool.tile([P, TOK], F32, name=f"xres{i}", tag=f"xres{i}")
                    nc.sync.dma_start(out=xr[:], in_=xT_d.ap()[i * P:(i + 1) * P, :].bitcast(F32))
                    xres.append(xr)
                wp = [wrow(wpool, wprojT_d, k, 0, C, f"wp{k}") for k in range(CT)]
                for t in range(CT):
                    pp_t = pp.tile([P, TOK], F32, name=f"pp{t}", tag="pp")
                    for k in range(CT):
                        for cc in range(TOK // QCH):
                            cs = slice(cc * QCH, (cc + 1) * QCH)
                            nc.tensor.matmul(pp_t[:, cs], wp[k][:, t * P:(t + 1) * P],
                                             ctxf[k][:, cs],
                                             start=(k == 0), stop=(k == CT - 1))
                    g = spool.tile([P, TOK], F32, name=f"g{t}", tag="g")
                    nc.scalar.activation(g[:], pp_t[:], AF.Identity,
                                         bias=projbg_sb[:, t:t + 1], scale=g1_sb[:, t:t + 1])
                    nc.vector.tensor_add(x1[t][:], g[:], xres[t][:])
                    if debug:
                        nc.sync.dma_start(out=dbg_x1_d.ap()[t * P:(t + 1) * P, :],
                                          in_=x1[t][:].bitcast(F32))

            _mark("p5_proj")
            # ================= P6: LN2 =================
            cm_n2 = tc.tile_pool(name="life_n2", bufs=1)
            pool_n2 = cm_n2.__enter__()
            n2i = [pool_n2.tile([P, TOK], BF16, name=f"n2i{i}", tag=f"n2i{i}")
                   for i in range(CT)]
            n2t = [pool_n2.tile([P, MT], BF16, name=f"n2t{i}", tag=f"n2t{i}")
                   for i in range(CT)]
            with tc.tile_pool(name="p6", bufs=2) as scratch, \
                 tc.tile_pool(name="p6psum", bufs=1, space="PSUM") as psump:
                layernorm(x1, TOK, n2iw_sb, 0, n2ib_sb, n2i, scratch, psump)
            with tc.tile_pool(name="p6b", bufs=2) as scratch, \
                 tc.tile_pool(name="p6bpsum", bufs=1, space="PSUM") as psump:
                layernorm(x1, MT, n2tw_sb, 0, n2tb_sb, n2t, scratch, psump)

            _mark("p6_ln2")
            # ================= P7: MLP image branch (hidden-chunked) =================
            HCH = HID // 2
            cm_mlp = tc.tile_pool(name="life_mlp", bufs=1, side="right")
            pool_mlp = cm_mlp.__enter__()
            mlp = [pool_mlp.tile([P, TOK], F32, name=f"mlp{i}", tag=f"mlp{i}")
                   for i in range(CT)]
            if skip_mlp:
                for i in range(CT):
                    nc.vector.tensor_copy(out=mlp[i][:],
                                          in_=ones_f[:, :].broadcast_to([P, TOK]))
            with tc.tile_pool(name="p7h", bufs=1) as hpool, \
                 tc.tile_pool(name="p7w", bufs=1) as wpool, \
                 tc.tile_pool(name="p7psum", bufs=2, space="PSUM") as pp:
                h1 = [hpool.tile([P, TOK], BF16, name=f"h1_{j}", tag=f"h1_{j}")
                      for j in range(HTC)]
                for hc in range(0 if skip_mlp else HID // HCH):
                    wf1 = [wrow(wpool, wfc1iT_d, k, hc * HCH, (hc + 1) * HCH,
                                f"wf1_{k}", BF16) for k in range(CT)]
                    for ht in range(HTC):
                        htg = hc * HTC + ht
                        ph = pp.tile([P, TOK], F32, name=f"ph{hc}_{ht}", tag="ph")
                        for k in range(CT):
                            for cc in range(TOK // QCH):
                                cs = slice(cc * QCH, (cc + 1) * QCH)
                                nc.tensor.matmul(ph[:, cs], wf1[k][:, ht * P:(ht + 1) * P],
                                                 n2i[k][:, cs],
                                                 start=(k == 0), stop=(k == CT - 1))
                        nc.scalar.activation(h1[ht][:], ph[:], AF.Gelu,
                                             bias=bfc1i_sb[:, htg:htg + 1])
                    wf2 = [wrow(wpool, wfc2iT_d, hc * HTC + k2, 0, C,
                                f"wf2_{k2}", BF16) for k2 in range(HTC)]
                    for t in range(CT):
                        po = pp.tile([P, TOK], F32, name=f"po{hc}_{t}", tag="po")
                        for k2 in range(HTC):
                            for cc in range(TOK // QCH):
                                cs = slice(cc * QCH, (cc + 1) * QCH)
                                nc.tensor.matmul(po[:, cs], wf2[k2][:, t * P:(t + 1) * P],
                                                 h1[k2][:, cs],
                                                 start=(k2 == 0), stop=(k2 == HTC - 1))
                        if hc == 0:
                            nc.scalar.activation(mlp[t][:], po[:], AF.Identity,
                                                 bias=g2b2i_sb[:, t:t + 1],
                                                 scale=g2_sb[:, t:t + 1])
                        else:
                            nc.vector.scalar_tensor_tensor(mlp[t][:], po[:],
                                                           g2_sb[:, t:t + 1],
                                                           mlp[t][:], ALU.mult, ALU.add)

            _mark("p7_mlp")
            # ================= P7t: touch branch + blend =================
            with tc.tile_pool(name="p7th", bufs=1) as hpool, \
                 tc.tile_pool(name="p7t", bufs=2) as spool, \
                 tc.tile_pool(name="p7tw", bufs=1) as wpool, \
                 tc.tile_pool(name="p7tpsum", bufs=4, space="PSUM") as pp:
                h1t = [hpool.tile([P, MT], BF16, name=f"h1t{j}", tag=f"h1t{j}")
                       for j in range(HID // P)]
                for htg in range(0 if skip_mlp else 2):
                    wf1t = [wrow(wpool, wfc1tT_d, k, htg * HCH, (htg + 1) * HCH,
                                 f"wf1t_{k}", BF16) for k in range(CT)]
                    for htl in range(HTC):
                        ht = htg * HTC + htl
                        ph = pp.tile([P, MT], F32, name=f"pht{ht}", tag="pht")
                        for k in range(CT):
                            nc.tensor.matmul(ph[:], wf1t[k][:, htl * P:(htl + 1) * P],
                                             n2t[k][:], start=(k == 0), stop=(k == CT - 1))
                        nc.scalar.activation(h1t[ht][:], ph[:], AF.Gelu,
                                             bias=bfc1t_sb[:, ht:ht + 1])
                wf2t = [wrow(wpool, wfc2tT_d, k2, 0, C, f"wf2t_{k2}", BF16)
                        for k2 in range(0 if skip_mlp else HID // P)]
                for t in range(0 if skip_mlp else CT):
                    po = pp.tile([P, MT], F32, name=f"pot{t}", tag="pot")
                    for k2 in range(HID // P):
                        nc.tensor.matmul(po[:], wf2t[k2][:, t * P:(t + 1) * P],
                                         h1t[k2][:],
                                         start=(k2 == 0), stop=(k2 == HID // P - 1))
                    mt_ = spool.tile([P, MT], F32, name=f"mt{t}", tag="mt")
                    nc.scalar.activation(mt_[:], po[:], AF.Identity,
                                         bias=g2b2t_sb[:, t:t + 1], scale=g2_sb[:, t:t + 1])
                    d = spool.tile([P, MT], F32, name=f"d{t}", tag="d")
                    nc.vector.tensor_sub(d[:], mt_[:], mlp[t][:, 0:MT])
                    nc.vector.scalar_tensor_tensor(mlp[t][:, 0:MT], d[:], sel_sb[:, 0:1],
                                                   mlp[t][:, 0:MT], ALU.mult, ALU.add)
            cm_n2.__exit__(None, None, None)

            _mark("p7t_touch")
            # ================= P8: final residual + out =================
            with tc.tile_pool(name="p8", bufs=2) as spool:
                for t in range(CT):
                    ot = spool.tile([P, TOK], F32, name=f"ot{t}", tag="ot")
                    nc.vector.tensor_add(ot[:], x1[t][:].bitcast(F32), mlp[t][:])
                    nc.sync.dma_start(out=outT_d.ap()[t * P:(t + 1) * P, :], in_=ot[:])
                    if debug:
                        nc.sync.dma_start(out=dbg_mlp_d.ap()[t * P:(t + 1) * P, :],
                                          in_=mlp[t][:])
            cm_mlp.__exit__(None, None, None)
            cm_x1.__exit__(None, None, None)

    _mark("p8_out")
    nc.compile()
    return nc


# ======================= host side =======================

def _cols(v):
    """[K*P]-vector -> [P, K] array (column k = slice k of the vector)."""
    return np.ascontiguousarray(np.asarray(v, np.float32).reshape(-1, P).T)


def prep_inputs(x, mask, relative_position_bias, norm1_w, norm1_b, qkv_w, q_bias,
                v_bias, proj_w, proj_b, gamma_1, gamma_2, norm2t_w, norm2t_b,
                t_fc1_w, t_fc1_b, t_fc2_w, t_fc2_b, norm2i_w, norm2i_b,
                i_fc1_w, i_fc1_b, i_fc2_w, i_fc2_b):
    f = lambda a: np.asarray(a, np.float32)
    x = f(x)
    rpb = f(relative_position_bias)
    mask = np.asarray(mask)

    qkv_w = f(qkv_w)
    wq = qkv_w[0:C] * SCALE
    wk = qkv_w[C:2 * C]
    wv = qkv_w[2 * C:3 * C]
    qb = f(q_bias) * SCALE
    vb = f(v_bias)
    blocks, bias_blocks = [], []
    for j in range(NCORE):
        r = slice(j * 2 * HD, (j + 1) * 2 * HD)
        blocks += [wq[r], wk[r], wv[r]]
        bias_blocks += [qb[r], np.zeros(2 * HD, np.float32), vb[r]]
    wqkvT = np.ascontiguousarray(np.concatenate(blocks, axis=0).T)  # [C, 3C]
    qkvb = np.concatenate(bias_blocks)                              # [3C]

    erpbT_all = np.exp(rpb).transpose(0, 2, 1)                      # [H, k, q]
    xT = np.ascontiguousarray(x.reshape(B * N, C).T)                # [C, 8192]
    g1 = f(gamma_1)
    g2 = f(gamma_2)

    maskf = mask.astype(np.float32)                                 # [B, N]
    maskm = np.ascontiguousarray(maskf.reshape(B * KT, P).T)        # [P, B*KT]

    common = dict(
        wqkvT=wqkvT, qkvb=_cols(qkvb), maskm=maskm,
        wprojT=np.ascontiguousarray(f(proj_w).T),
        projbg=_cols(g1 * f(proj_b)), g1=_cols(g1),
        n1w=_cols(norm1_w), n1b=_cols(norm1_b),
        n2iw=_cols(norm2i_w), n2ib=_cols(norm2i_b),
        n2tw=_cols(norm2t_w), n2tb=_cols(norm2t_b),
        wfc1iT=np.ascontiguousarray(f(i_fc1_w).T).astype(ml_dtypes.bfloat16),
        bfc1i=_cols(i_fc1_b),
        wfc2iT=np.ascontiguousarray(f(i_fc2_w).T).astype(ml_dtypes.bfloat16),
        g2b2i=_cols(g2 * f(i_fc2_b)),
        wfc1tT=np.ascontiguousarray(f(t_fc1_w).T).astype(ml_dtypes.bfloat16),
        bfc1t=_cols(t_fc1_b),
        wfc2tT=np.ascontiguousarray(f(t_fc2_w).T).astype(ml_dtypes.bfloat16),
        g2b2t=_cols(g2 * f(t_fc2_b)),
        g2=_cols(g2),
    )

    in_maps = []
    for c in range(NCORE):
        im = dict(common)
        im["xT"] = np.ascontiguousarray(xT[:, c * TOK:(c + 1) * TOK])
        im["erpbT"] = np.ascontiguousarray(
            erpbT_all[c * HP:(c + 1) * HP]).astype(ml_dtypes.bfloat16)
        im["sel"] = np.full((P, 1), 1.0 if (c % 2 == 0) else 0.0, np.float32)
        in_maps.append(im)
    return in_maps


_NC_CACHE = {}


def get_nc(debug=False, apply_mask=False):
    key = (debug, apply_mask)
    if key not in _NC_CACHE:
        _NC_CACHE[key] = build_kernel(debug=debug, apply_mask=apply_mask)
    return _NC_CACHE[key]


def run(inputs, debug=False):
    am = not np.all(np.asarray(inputs["mask"]) == 1)
    nc = get_nc(debug=debug, apply_mask=am)
    in_maps = prep_inputs(**inputs)
    return run_bass_kernel_spmd(nc, in_maps, core_ids=list(range(NCORE)))


def kernel(**inputs):
    res = run(inputs, debug=False)
    outT = np.concatenate([res.results[c]["outT"] for c in range(NCORE)], axis=1)
    return np.ascontiguousarray(outT.T).reshape(B, N, C).astype(np.float32)



# revision 10
# speedup vs baseline: 1.5772x; 1.5772x over previous
"""Trainium2 Bass kernel for nn_Block_15118284882621 (dense transformer block).

Sharding (8 NeuronCores, single chip):
  - qkv + MLP: token-parallel (1024 tokens/core, channel-major layout)
  - attention: head-pair-parallel (2 heads/core, all 8192 tokens)
  - connected by 4 AllToAlls (q, k, v: 2 MB bf16; ctx: 1 MB fp8), fired
    kind-by-kind as soon as each projection completes so they overlap the
    rest of qkv and the start of attention.
  - touch-token MLP branch: hidden-sharded 8 ways (AllGather tokens,
    AllReduce partial outputs) instead of 8x-replicated weights.

Matmul dtypes: fp8e4 DoubleRow (2 fp8/cell) for qkv/proj/MLP weights
(x16 host-side scale, folded back in the epilogues); bf16 for attention
scores and attn@v. Softmax pipeline: scores fp32 in PSUM (b-pair tiles,
double-buffered so PE/Act/DVE overlap), exp -> bf16, multiplicative
exp(rpb) bias (host-precomputed bf16) applied at DVE 2x bf16 rate
(part on Pool); denominator from a ones-column in the attn@v matmul.
"""

import numpy as np
import ml_dtypes

import concourse.bass as bass
from concourse import bacc
import concourse.mybir as mybir
import concourse.tile as tile
from concourse.bass_utils import run_bass_kernel_spmd
from concourse.masks import make_identity

B, N, C, H, MT = 4, 2048, 1024, 16, 40
HD = C // H            # 64
HID = 4 * C            # 4096
SCALE = HD ** -0.5
EPS = 1e-5
NCORE = 8
TOK = B * N // NCORE   # 1024 tokens per core
HP = H // NCORE        # 2 heads per core
P = 128
F32 = mybir.dt.float32
F32R = mybir.dt.float32r
BF16 = mybir.dt.bfloat16
F8 = mybir.dt.float8e4
AF = mybir.ActivationFunctionType
ALU = mybir.AluOpType
DR = mybir.MatmulPerfMode.DoubleRow
RG = [[0, 1, 2, 3, 4, 5, 6, 7]]

CT = C // P            # 8 channel tiles
KT = N // P            # 16 k tiles per batch
QCH = 512              # attention q-chunk width
HTC = 16               # hidden tiles per MLP chunk (chunk = 2048)
HSH = HID // NCORE     # 512: per-core hidden slice of the touch branch
WS = 16.0              # fp8 weight scale
CS = 64.0              # fp8 ctx scale

PHASE_MARKS = []

# bisect knobs (affect build only; host data layout unchanged)
import os as _os
USE_DR = _os.environ.get("K_NO_DR", "") == ""       # fp8 DoubleRow matmuls
SKIP_CC = _os.environ.get("K_SKIP_CC", "") != ""    # stub out collectives


def build_kernel(debug=False, apply_mask=True):
    nc = bacc.Bacc(None, target_bir_lowering=False)
    PHASE_MARKS.clear()

    def _mark(name):
        PHASE_MARKS.append((name, len(nc.inst_map)))

    def mm8(out, w, x, wk0, xk0, wcols, xcols, start, stop):
        """fp8 pair-of-ksubtiles matmul: DoubleRow or two plain matmuls."""
        if USE_DR:
            nc.tensor.matmul(out, w[:, wk0:wk0 + 2, wcols], x[:, xk0:xk0 + 2, xcols],
                             start=start, stop=stop, perf_mode=DR)
        else:
            nc.tensor.matmul(out, w[:, wk0, wcols], x[:, xk0, xcols],
                             start=start, stop=False)
            nc.tensor.matmul(out, w[:, wk0 + 1, wcols], x[:, xk0 + 1, xcols],
                             start=False, stop=stop)

    def collective(kind, op, ins, outs):
        if SKIP_CC:
            return
        nc.gpsimd.collective_compute(kind, op, ins=ins, outs=outs,
                                     replica_groups=RG)

    # ---------------- external I/O (per core) ----------------
    xT_d = nc.dram_tensor("xT", [C, TOK], F32R, kind="ExternalInput")
    wqkv8_d = nc.dram_tensor("wqkv8", [3, P, CT, C], F8, kind="ExternalInput")
    qkvb_d = nc.dram_tensor("qkvb", [P, 3 * CT], F32, kind="ExternalInput")
    erpbT_d = nc.dram_tensor("erpbT", [HP, N, N], BF16, kind="ExternalInput")
    maskm_d = nc.dram_tensor("maskm", [P, B * KT], F32, kind="ExternalInput")
    wproj8_d = nc.dram_tensor("wproj8", [P, CT, C], F8, kind="ExternalInput")
    projbg_d = nc.dram_tensor("projbg", [P, CT], F32, kind="ExternalInput")
    g1q_d = nc.dram_tensor("g1q", [P, CT], F32, kind="ExternalInput")
    n1w_d = nc.dram_tensor("n1w", [P, CT], F32, kind="ExternalInput")
    n1b_d = nc.dram_tensor("n1b", [P, CT], F32, kind="ExternalInput")
    n2iw_d = nc.dram_tensor("n2iw", [P, CT], F32, kind="ExternalInput")
    n2ib_d = nc.dram_tensor("n2ib", [P, CT], F32, kind="ExternalInput")
    n2tw_d = nc.dram_tensor("n2tw", [P, CT], F32, kind="ExternalInput")
    n2tb_d = nc.dram_tensor("n2tb", [P, CT], F32, kind="ExternalInput")
    wfc1i8_d = nc.dram_tensor("wfc1i8", [P, CT, HID], F8, kind="ExternalInput")
    bfc1i_d = nc.dram_tensor("bfc1i", [P, HID // P], F32, kind="ExternalInput")
    wfc2i8_d = nc.dram_tensor("wfc2i8", [P, HID // P, C], F8, kind="ExternalInput")
    g2b2i_d = nc.dram_tensor("g2b2i", [P, CT], F32, kind="ExternalInput")
    g2q_d = nc.dram_tensor("g2q", [P, CT], F32, kind="ExternalInput")
    g2f_d = nc.dram_tensor("g2f", [P, CT], F32, kind="ExternalInput")
    wfc1t_d = nc.dram_tensor("wfc1t", [P, CT, HSH], BF16, kind="ExternalInput")
    bfc1t_d = nc.dram_tensor("bfc1t", [P, HSH // P], F32, kind="ExternalInput")
    wfc2t_d = nc.dram_tensor("wfc2t", [P, HSH // P, C], BF16, kind="ExternalInput")
    g2b2t_d = nc.dram_tensor("g2b2t", [P, CT], F32, kind="ExternalInput")
    oh_d = nc.dram_tensor("oh", [P, B], F32, kind="ExternalInput")
    sel_d = nc.dram_tensor("sel", [P, 1], F32, kind="ExternalInput")

    outT_d = nc.dram_tensor("outT", [C, TOK], F32, kind="ExternalOutput")
    if debug:
        dbg_xn_d = nc.dram_tensor("dbg_xn", [C, TOK], F32, kind="ExternalOutput")
        dbg_ctx_d = nc.dram_tensor("dbg_ctx", [P, B * N], F32, kind="ExternalOutput")
        dbg_x1_d = nc.dram_tensor("dbg_x1", [C, TOK], F32, kind="ExternalOutput")
        dbg_mlp_d = nc.dram_tensor("dbg_mlp", [C, TOK], F32, kind="ExternalOutput")

    with tile.TileContext(nc) as tc:
        with tc.tile_pool(name="const", bufs=1) as const, \
             tc.tile_pool(name="dram", bufs=1, space="DRAM") as dram:

            ident_f = const.tile([P, P], F32, name="ident_f", tag="ident_f")
            make_identity(nc, ident_f)
            ident_bf = const.tile([P, P], BF16, name="ident_bf", tag="ident_bf")
            nc.vector.tensor_copy(out=ident_bf[:], in_=ident_f[:])
            ones_f = const.tile([P, 1], F32, name="ones_f", tag="ones_f")
            nc.vector.memset(ones_f[:], 1.0)
            ones_v = const.tile([P, 1], F32R, name="ones_v", tag="ones_v")
            nc.vector.tensor_copy(out=ones_v[:], in_=ones_f[:])
            ones_bf = const.tile([P, 1], BF16, name="ones_bf", tag="ones_bf")
            nc.vector.tensor_copy(out=ones_bf[:], in_=ones_f[:])

            def load_const(name, dten, cols):
                t = const.tile([P, cols], F32, name=name, tag=name)
                nc.sync.dma_start(out=t[:], in_=dten.ap())
                return t

            qkvb_sb = load_const("qkvb_sb", qkvb_d, 3 * CT)
            maskm_sb = load_const("maskm_sb", maskm_d, B * KT)
            projbg_sb = load_const("projbg_sb", projbg_d, CT)
            g1q_sb = load_const("g1q_sb", g1q_d, CT)
            n1w_sb = load_const("n1w_sb", n1w_d, CT)
            n1b_sb = load_const("n1b_sb", n1b_d, CT)
            n2iw_sb = load_const("n2iw_sb", n2iw_d, CT)
            n2ib_sb = load_const("n2ib_sb", n2ib_d, CT)
            n2tw_sb = load_const("n2tw_sb", n2tw_d, CT)
            n2tb_sb = load_const("n2tb_sb", n2tb_d, CT)
            bfc1i_sb = load_const("bfc1i_sb", bfc1i_d, HID // P)
            g2b2i_sb = load_const("g2b2i_sb", g2b2i_d, CT)
            g2q_sb = load_const("g2q_sb", g2q_d, CT)
            g2f_sb = load_const("g2f_sb", g2f_d, CT)
            bfc1t_sb = load_const("bfc1t_sb", bfc1t_d, HSH // P)
            g2b2t_sb = load_const("g2b2t_sb", g2b2t_d, CT)
            oh_sb = load_const("oh_sb", oh_d, B)
            sel_sb = load_const("sel_sb", sel_d, 1)

            # DRAM bounce buffers for collectives
            def dram_buf(name, shape, dtype=F32):
                return dram.tile(shape, dtype, name=name, tag=name)

            inq = dram_buf("inq", [NCORE, P, TOK], BF16)
            ink = dram_buf("ink", [NCORE, P, TOK], BF16)
            inv = dram_buf("inv", [NCORE, TOK, P], BF16)
            in2 = dram_buf("in2", [NCORE, P, TOK], F8)
            if SKIP_CC:
                outq, outk, outv, out2 = inq, ink, inv, in2
            else:
                outq = dram_buf("outq", [NCORE, P, TOK], BF16)
                outk = dram_buf("outk", [NCORE, P, TOK], BF16)
                outv = dram_buf("outv", [NCORE, TOK, P], BF16)
                out2 = dram_buf("out2", [NCORE, P, TOK], F8)
            in_t = dram_buf("in_t", [P, CT, MT], BF16)
            out_t = dram_buf("out_t", [NCORE, P, CT, MT], BF16)
            in_r = dram_buf("in_r", [CT, P, B * MT], BF16)
            out_r = dram_buf("out_r", [CT, P, B * MT], BF16)

            # lifetime-scoped static pools (entered/exited manually)
            cm_xt = tc.tile_pool(name="life_xt", bufs=1)
            pool_xt = cm_xt.__enter__()
            cm_xn = tc.tile_pool(name="life_xn", bufs=1, side="right")
            pool_xn = cm_xn.__enter__()
            xt = [pool_xt.tile([P, TOK], F32R, name=f"xt{i}", tag=f"xt{i}") for i in range(CT)]
            xn8 = pool_xn.tile([P, CT, TOK], F8, name="xn8", tag="xn8")

            # ---------- channel-major layernorm over the channel axis ----------
            def layernorm(src_tiles, cols, w_sb, b_sb, out_ap, scratch, psump):
                """src_tiles: CT SBUF tiles [P, >=cols] (F32R); out_ap(i) -> AP"""
                sums = psump.tile([1, cols], F32, name="ln_sums", tag="ln_sums")
                sumsq = psump.tile([1, cols], F32, name="ln_sumsq", tag="ln_sumsq")
                nchunk = (cols + QCH - 1) // QCH
                for i in range(CT):
                    sq = scratch.tile([P, cols], F32R, name=f"ln_sq{i}", tag="ln_sq")
                    nc.vector.tensor_mul(sq[:],
                                         src_tiles[i][:, 0:cols].bitcast(F32),
                                         src_tiles[i][:, 0:cols].bitcast(F32))
                    for cc in range(nchunk):
                        cs = slice(cc * QCH, min((cc + 1) * QCH, cols))
                        nc.tensor.matmul(sums[:, cs], ones_v[:], src_tiles[i][:, cs],
                                         start=(i == 0), stop=(i == CT - 1))
                        nc.tensor.matmul(sumsq[:, cs], ones_v[:], sq[:, cs],
                                         start=(i == 0), stop=(i == CT - 1))
                mu = scratch.tile([1, cols], F32, name="ln_mu", tag="ln_mu")
                nc.vector.tensor_scalar_mul(mu[:], sums[:], 1.0 / C)
                msq = scratch.tile([1, cols], F32, name="ln_msq", tag="ln_msq")
                nc.vector.tensor_scalar_mul(msq[:], sumsq[:], 1.0 / C)
                var = scratch.tile([1, cols], F32, name="ln_var", tag="ln_var")
                nc.vector.scalar_tensor_tensor(var[:], mu[:], -1.0, mu[:],
                                               ALU.mult, ALU.mult)
                nc.vector.tensor_add(var[:], var[:], msq[:])
                nc.vector.tensor_scalar_add(var[:], var[:], EPS)
                sd = scratch.tile([1, cols], F32, name="ln_sd", tag="ln_sd")
                nc.scalar.activation(sd[:], var[:], AF.Sqrt)
                rstd = scratch.tile([1, cols], F32, name="ln_rstd", tag="ln_rstd")
                nc.vector.reciprocal(rstd[:], sd[:])
                mub = scratch.tile([P, cols], F32, name="ln_mub", tag="ln_mub")
                nc.gpsimd.partition_broadcast(mub[:], mu[:], channels=P)
                rstdb = scratch.tile([P, cols], F32, name="ln_rstdb", tag="ln_rstdb")
                nc.gpsimd.partition_broadcast(rstdb[:], rstd[:], channels=P)
                for i in range(CT):
                    t1 = scratch.tile([P, cols], F32, name=f"ln_t1_{i}", tag="ln_t1")
                    nc.vector.tensor_sub(t1[:], src_tiles[i][:, 0:cols].bitcast(F32), mub[:])
                    t2 = scratch.tile([P, cols], F32, name=f"ln_t2_{i}", tag="ln_t2")
                    nc.vector.tensor_mul(t2[:], t1[:], rstdb[:])
                    nc.vector.scalar_tensor_tensor(
                        out_ap(i), t2[:], w_sb[:, i:i + 1],
                        b_sb[:, i:i + 1].broadcast_to([P, cols]), ALU.mult, ALU.add)

            # ================= P0: load x, LN1 =================
            with tc.tile_pool(name="p0", bufs=2) as scratch, \
                 tc.tile_pool(name="p0psum", bufs=1, space="PSUM") as psump:
                for i in range(CT):
                    nc.sync.dma_start(out=xt[i][:], in_=xT_d.ap()[i * P:(i + 1) * P, :])
                layernorm(xt, TOK, n1w_sb, n1b_sb, lambda i: xn8[:, i, :],
                          scratch, psump)
                if debug:
                    for i in range(CT):
                        dx = scratch.tile([P, TOK], F32, name=f"dbgx{i}", tag="dbgx")
                        nc.vector.tensor_copy(out=dx[:], in_=xn8[:, i, :])
                        nc.sync.dma_start(out=dbg_xn_d.ap()[i * P:(i + 1) * P, :], in_=dx[:])

            _mark("p0_ln1")
            # ======== P1: qkv (fp8 DoubleRow), kind-major; A2A per kind ========
            with tc.tile_pool(name="p1w", bufs=2) as wpool, \
                 tc.tile_pool(name="p1s", bufs=3) as spool, \
                 tc.tile_pool(name="p1v", bufs=2) as vpool, \
                 tc.tile_pool(name="p1psum", bufs=3, space="PSUM") as pp, \
                 tc.tile_pool(name="p1psumT", bufs=2, space="PSUM") as ppt:
                for kind in range(3):               # 0=q, 1=k, 2=v
                    wk8 = wpool.tile([P, CT, C], F8, name=f"wk8_{kind}", tag="wk8")
                    nc.sync.dma_start(out=wk8[:], in_=wqkv8_d.ap()[kind])
                    for j in range(NCORE):
                        t = kind * NCORE + j
                        pq = pp.tile([P, TOK], F32, name=f"pq{t}", tag="pq")
                        for cc in range(TOK // QCH):
                            cs = slice(cc * QCH, (cc + 1) * QCH)
                            for kp in range(CT // 2):
                                mm8(pq[:, cs], wk8, xn8, 2 * kp, 2 * kp,
                                    slice(j * P, (j + 1) * P), cs,
                                    kp == 0, kp == CT // 2 - 1)
                        qv = spool.tile([P, TOK], BF16, name=f"qv{t}", tag="qv")
                        nc.vector.scalar_tensor_tensor(
                            qv[:], pq[:], 1.0 / WS,
                            qkvb_sb[:, t:t + 1].broadcast_to([P, TOK]),
                            ALU.mult, ALU.add)
                        if kind == 0:
                            nc.sync.dma_start(out=inq[j], in_=qv[:])
                        elif kind == 1:
                            nc.sync.dma_start(out=ink[j], in_=qv[:])
                        else:
                            vtb = vpool.tile([P, TOK // P, P], BF16,
                                             name=f"vtb{j}", tag="vtb")
                            for c8 in range(TOK // P):
                                tp = ppt.tile([P, P], BF16, name=f"tp{t}_{c8}", tag="tp")
                                nc.tensor.transpose(tp[:], qv[:, c8 * P:(c8 + 1) * P],
                                                    ident_bf[:])
                                nc.vector.tensor_copy(out=vtb[:, c8, :], in_=tp[:])
                            nc.sync.dma_start(
                                out=inv[j].rearrange("(c p) d -> p c d", p=P),
                                in_=vtb[:])
                    if kind == 0:
                        collective("AllToAll", ALU.bypass, [inq[:]], [outq[:]])
                    elif kind == 1:
                        collective("AllToAll", ALU.bypass, [ink[:]], [outk[:]])
                    else:
                        collective("AllToAll", ALU.bypass, [inv[:]], [outv[:]])
            cm_xn.__exit__(None, None, None)  # xn8 dead after qkv
            cm_xt.__exit__(None, None, None)  # x reloaded from DRAM at P5

            _mark("p1_qkv")
            # ================= P3: attention =================
            cm_ctx = tc.tile_pool(name="life_ctx", bufs=1, side="right")
            pool_ctx = cm_ctx.__enter__()
            ctx_sb = pool_ctx.tile([P, B * N], F8, name="ctx_sb", tag="ctx_sb")
            with tc.tile_pool(name="p3qk", bufs=1) as qkpool, \
                 tc.tile_pool(name="p3va", bufs=1) as vapool, \
                 tc.tile_pool(name="p3s", bufs=4) as spool, \
                 tc.tile_pool(name="p3er", bufs=4) as erpool, \
                 tc.tile_pool(name="p3ps", bufs=2, space="PSUM") as psS, \
                 tc.tile_pool(name="p3pc", bufs=1, space="PSUM") as psC:
                for h in range(HP):
                    # paired q/k tiles: rows 0:64 = batch 2j, 64:128 = batch 2j+1
                    qh2, kh2 = [], []
                    for j in range(B // 2):
                        qt = qkpool.tile([P, N], BF16, name=f"qh{h}_{j}", tag=f"qh{h}_{j}")
                        kt_ = qkpool.tile([P, N], BF16, name=f"kh{h}_{j}", tag=f"kh{h}_{j}")
                        for bb in range(2):
                            for half in range(2):
                                srcc = 2 * (2 * j + bb) + half
                                dst = slice(half * TOK, (half + 1) * TOK)
                                nc.sync.dma_start(
                                    out=qt[bb * HD:(bb + 1) * HD, dst],
                                    in_=outq[srcc][h * HD:(h + 1) * HD, :])
                                nc.sync.dma_start(
                                    out=kt_[bb * HD:(bb + 1) * HD, dst],
                                    in_=outk[srcc][h * HD:(h + 1) * HD, :])
                        qh2.append(qt)
                        kh2.append(kt_)

                    def qk_ap(lst, b, cols):
                        t = lst[b // 2]
                        r = (b % 2) * HD
                        return t[r:r + HD, cols]

                    # v tiles: [128 tok-part, 8 k-chunks, 65] (col 64 = ones)
                    vbig = {}
                    for b in range(B):
                        for half in range(2):
                            vt_ = vapool.tile([P, KT // 2, HD + 1], BF16,
                                              name=f"vb{h}_{b}_{half}",
                                              tag=f"vb{h}_{b}_{half}")
                            nc.sync.dma_start(
                                out=vt_[:, :, 0:HD],
                                in_=outv[2 * b + half].rearrange(
                                    "(c p) d -> p c d", p=P)[:, :, h * HD:(h + 1) * HD])
                            nc.vector.tensor_copy(
                                out=vt_[:, :, HD:HD + 1],
                                in_=ones_bf[:, None, :].broadcast_to([P, KT // 2, 1]))
                            if apply_mask:
                                for c8 in range(KT // 2):
                                    mi = b * KT + half * (KT // 2) + c8
                                    nc.vector.tensor_scalar_mul(
                                        vt_[:, c8, :], vt_[:, c8, :],
                                        maskm_sb[:, mi:mi + 1])
                            vbig[(b, half)] = vt_

                    for qq in range(N // QCH):
                        qs = slice(qq * QCH, (qq + 1) * QCH)
                        ctxp = [psC.tile([HD + 1, QCH], F32,
                                         name=f"ctxp{h}_{qq}_{b}", tag=f"ctxp{b}")
                                for b in range(B)]
                        for kt in range(KT):
                            er = erpool.tile([P, QCH], BF16, name=f"er{h}_{qq}_{kt}", tag="er")
                            nc.sync.dma_start(out=er[:],
                                              in_=erpbT_d.ap()[h, kt * P:(kt + 1) * P, qs])
                            for bp in range(2):
                                sp = psS.tile([P, 2, QCH], F32,
                                              name=f"sp{h}_{qq}_{kt}_{bp}", tag="sp")
                                for bi in range(2):
                                    b = 2 * bp + bi
                                    nc.tensor.matmul(
                                        sp[:, bi, :],
                                        qk_ap(kh2, b, slice(kt * P, (kt + 1) * P)),
                                        qk_ap(qh2, b, qs), start=True, stop=True,
                                        skip_group_check=True)
                                tmp = spool.tile([P, 2, QCH], BF16,
                                                 name=f"tm{h}_{qq}_{kt}_{bp}", tag="tmp")
                                nc.scalar.activation(tmp[:], sp[:], AF.Exp)
                                pkt = spool.tile([P, 2, QCH], BF16,
                                                 name=f"pk{h}_{qq}_{kt}_{bp}", tag="pkt")
                                erb = er[:, None, :].broadcast_to([P, 2, QCH])
                                if (2 * kt + bp) % 8 < 3:
                                    nc.gpsimd.tensor_mul(pkt[:], tmp[:], erb)
                                else:
                                    nc.vector.tensor_mul(pkt[:], tmp[:], erb)
                                for bi in range(2):
                                    b = 2 * bp + bi
                                    nc.tensor.matmul(
                                        ctxp[b][:],
                                        vbig[(b, kt // (KT // 2))][:, kt % (KT // 2), :],
                                        pkt[:, bi, :],
                                        start=(kt == 0), stop=(kt == KT - 1))
                        for b in range(B):
                            rec = spool.tile([1, QCH], F32, name=f"rec{h}_{qq}_{b}", tag="rec")
                            nc.vector.reciprocal(rec[:], ctxp[b][HD:HD + 1, :])
                            nc.vector.tensor_scalar_mul(rec[:], rec[:], CS)
                            recb = spool.tile([HD, QCH], F32,
                                              name=f"recb{h}_{qq}_{b}", tag="recb")
                            nc.gpsimd.partition_broadcast(recb[:], rec[:], channels=HD)
                            nc.vector.tensor_mul(
                                ctx_sb[h * HD:(h + 1) * HD,
                                       b * N + qq * QCH: b * N + (qq + 1) * QCH],
                                ctxp[b][0:HD, :], recb[:])

            _mark("p3_attn")
            # ================= P4: ctx A2A (fp8) =================
            for j in range(NCORE):
                nc.sync.dma_start(out=in2[j], in_=ctx_sb[:, j * TOK:(j + 1) * TOK])
            collective("AllToAll", ALU.bypass, [in2[:]], [out2[:]])
            if debug:
                with tc.tile_pool(name="dbgc", bufs=1) as dpool:
                    dc = dpool.tile([P, B * N], F32, name="dbgctx", tag="dbgctx")
                    nc.vector.tensor_copy(out=dc[:], in_=ctx_sb[:])
                    nc.vector.tensor_scalar_mul(dc[:], dc[:], 1.0 / CS)
                    nc.sync.dma_start(out=dbg_ctx_d.ap(), in_=dc[:])
            cm_ctx.__exit__(None, None, None)

            _mark("p4_a2a")
            # ================= P5: proj (fp8 DR) + residual =================
            cm_x1 = tc.tile_pool(name="life_x1", bufs=1, side="right")
            pool_x1 = cm_x1.__enter__()
            x1 = [pool_x1.tile([P, TOK], F32R, name=f"x1_{i}", tag=f"x1_{i}")
                  for i in range(CT)]
            with tc.tile_pool(name="p5c", bufs=1) as cpool, \
                 tc.tile_pool(name="p5", bufs=2) as spool, \
                 tc.tile_pool(name="p5psum", bufs=3, space="PSUM") as pp:
                ctxf8 = cpool.tile([P, CT, TOK], F8, name="ctxf8", tag="ctxf8")
                nc.sync.dma_start(out=ctxf8[:],
                                  in_=out2[:].rearrange("s p t -> p s t"))
                wp8 = cpool.tile([P, CT, C], F8, name="wp8", tag="wp8")
                nc.sync.dma_start(out=wp8[:], in_=wproj8_d.ap())
                xres = []
                for i in range(CT):
                    xr = cpool.tile([P, TOK], F32, name=f"xres{i}", tag=f"xres{i}")
                    nc.sync.dma_start(out=xr[:],
                                      in_=xT_d.ap()[i * P:(i + 1) * P, :].bitcast(F32))
                    xres.append(xr)
                for t in range(CT):
                    pp_t = pp.tile([P, TOK], F32, name=f"pp{t}", tag="pp")
                    for cc in range(TOK // QCH):
                        cs = slice(cc * QCH, (cc + 1) * QCH)
                        for kp in range(CT // 2):
                            mm8(pp_t[:, cs], wp8, ctxf8, 2 * kp, 2 * kp,
                                slice(t * P, (t + 1) * P), cs,
                                kp == 0, kp == CT // 2 - 1)
                    g = spool.tile([P, TOK], F32, name=f"g{t}", tag="g")
                    nc.scalar.activation(g[:], pp_t[:], AF.Identity,
                                         bias=projbg_sb[:, t:t + 1],
                                         scale=g1q_sb[:, t:t + 1])
                    nc.vector.tensor_add(x1[t][:], g[:], xres[t][:])
                    if debug:
                        nc.sync.dma_start(out=dbg_x1_d.ap()[t * P:(t + 1) * P, :],
                                          in_=x1[t][:].bitcast(F32))

            _mark("p5_proj")
            # ================= P6: LN2 (image fp8, touch bf16 + AllGather) ====
            cm_n2 = tc.tile_pool(name="life_n2", bufs=1)
            pool_n2 = cm_n2.__enter__()
            n2i8 = pool_n2.tile([P, CT, TOK], F8, name="n2i8", tag="n2i8")
            n2t = pool_n2.tile([P, CT, MT], BF16, name="n2t", tag="n2t")
            with tc.tile_pool(name="p6b", bufs=2) as scratch, \
                 tc.tile_pool(name="p6bpsum", bufs=1, space="PSUM") as psump:
                layernorm(x1, MT, n2tw_sb, n2tb_sb, lambda i: n2t[:, i, :],
                          scratch, psump)
            nc.sync.dma_start(out=in_t[:], in_=n2t[:])
            collective("AllGather", ALU.bypass, [in_t[:]], [out_t[:]])
            with tc.tile_pool(name="p6", bufs=2) as scratch, \
                 tc.tile_pool(name="p6psum", bufs=1, space="PSUM") as psump:
                layernorm(x1, TOK, n2iw_sb, n2ib_sb, lambda i: n2i8[:, i, :],
                          scratch, psump)

            _mark("p6_ln2")
            # ========== P7: MLP image branch (fp8 DR, hidden-chunked), ========
            # ========== touch branch sandwiched between the two chunks ========
            HCH = HID // 2
            cm_mlp = tc.tile_pool(name="life_mlp", bufs=1, side="right")
            pool_mlp = cm_mlp.__enter__()
            mlp = [pool_mlp.tile([P, TOK], F32, name=f"mlp{i}", tag=f"mlp{i}")
                   for i in range(CT)]

            def touch_branch(tpool, twpool, tps):
                n2tg = twpool.tile([P, CT, B * MT], BF16, name="n2tg", tag="n2tg")
                for j in range(B):
                    nc.sync.dma_start(out=n2tg[:, :, j * MT:(j + 1) * MT],
                                      in_=out_t[2 * j])
                w1t = twpool.tile([P, CT, HSH], BF16, name="w1t", tag="w1t")
                nc.sync.dma_start(out=w1t[:], in_=wfc1t_d.ap())
                h1t = twpool.tile([P, HSH // P, B * MT], BF16, name="h1t", tag="h1t")
                for ht in range(HSH // P):
                    pht = tps.tile([P, B * MT], F32, name=f"pht{ht}", tag="pt")
                    for k in range(CT):
                        nc.tensor.matmul(pht[:], w1t[:, k, ht * P:(ht + 1) * P],
                                         n2tg[:, k, :], start=(k == 0),
                                         stop=(k == CT - 1))
                    nc.scalar.activation(h1t[:, ht, :], pht[:], AF.Gelu,
                                         bias=bfc1t_sb[:, ht:ht + 1])
                w2t = twpool.tile([P, HSH // P, C], BF16, name="w2t", tag="w2t")
                nc.sync.dma_start(out=w2t[:], in_=wfc2t_d.ap())
                for t in range(CT):
                    pot = tps.tile([P, B * MT], F32, name=f"pot{t}", tag="pt")
                    for k2 in range(HSH // P):
                        nc.tensor.matmul(pot[:], w2t[:, k2, t * P:(t + 1) * P],
                                         h1t[:, k2, :], start=(k2 == 0),
                                         stop=(k2 == HSH // P - 1))
                    ptc = tpool.tile([P, B * MT], BF16, name=f"ptc{t}", tag="ptc")
                    nc.vector.tensor_copy(out=ptc[:], in_=pot[:])
                    nc.sync.dma_start(out=in_r[t], in_=ptc[:])
                collective("AllReduce", ALU.add, [in_r[:]], [out_r[:]])

            with tc.tile_pool(name="p7h", bufs=1) as hpool, \
                 tc.tile_pool(name="p7w", bufs=2) as wpool, \
                 tc.tile_pool(name="p7t", bufs=2) as tpool, \
                 tc.tile_pool(name="p7tw", bufs=1) as twpool, \
                 tc.tile_pool(name="p7tpsum", bufs=2, space="PSUM") as tps, \
                 tc.tile_pool(name="p7psum", bufs=2, space="PSUM") as pp:
                h18 = hpool.tile([P, HID // P, TOK], F8, name="h18", tag="h18")
                for hc in range(HID // HCH):
                    wf1 = wpool.tile([P, CT, HCH], F8, name=f"wf1_{hc}", tag="wf1")
                    nc.sync.dma_start(out=wf1[:],
                                      in_=wfc1i8_d.ap()[:, :, hc * HCH:(hc + 1) * HCH])
                    for ht in range(HTC):
                        htg = hc * HTC + ht
                        ph = pp.tile([P, TOK], F32, name=f"ph{hc}_{ht}", tag="pmm")
                        for cc in range(TOK // QCH):
                            cs = slice(cc * QCH, (cc + 1) * QCH)
                            for kp in range(CT // 2):
                                mm8(ph[:, cs], wf1, n2i8, 2 * kp, 2 * kp,
                                    slice(ht * P, (ht + 1) * P), cs,
                                    kp == 0, kp == CT // 2 - 1)
                        nc.scalar.activation(h18[:, htg, :], ph[:], AF.Gelu,
                                             bias=bfc1i_sb[:, htg:htg + 1],
                                             scale=1.0 / WS)
                    wf2 = wpool.tile([P, HTC, C], F8, name=f"wf2_{hc}", tag="wf2")
                    nc.sync.dma_start(out=wf2[:],
                                      in_=wfc2i8_d.ap()[:, hc * HTC:(hc + 1) * HTC, :])
                    for t in range(CT):
                        po = pp.tile([P, TOK], F32, name=f"po{hc}_{t}", tag="pmm")
                        for cc in range(TOK // QCH):
                            cs = slice(cc * QCH, (cc + 1) * QCH)
                            for kp in range(HTC // 2):
                                mm8(po[:, cs], wf2, h18, 2 * kp, hc * HTC + 2 * kp,
                                    slice(t * P, (t + 1) * P), cs,
                                    kp == 0, kp == HTC // 2 - 1)
                        if hc == 0:
                            nc.scalar.activation(mlp[t][:], po[:], AF.Identity,
                                                 bias=g2b2i_sb[:, t:t + 1],
                                                 scale=g2q_sb[:, t:t + 1])
                        else:
                            nc.vector.scalar_tensor_tensor(mlp[t][:], po[:],
                                                           g2q_sb[:, t:t + 1],
                                                           mlp[t][:], ALU.mult, ALU.add)
                    if hc == 0:
                        # touch branch rides here: AllGather has landed by now,
                        # AllReduce overlaps the second image chunk
                        touch_branch(tpool, twpool, tps)
            cm_n2.__exit__(None, None, None)

            _mark("p7_mlp")
            # ====== P7t-b: blend reduced touch output into mlp[:, :MT] ======
            with tc.tile_pool(name="p7b", bufs=2) as bpool:
                for t in range(CT):
                    rsum = bpool.tile([P, B * MT], BF16, name=f"rs{t}", tag="rs")
                    nc.sync.dma_start(out=rsum[:], in_=out_r[t])
                    acc = bpool.tile([P, MT], F32, name=f"acc{t}", tag="acc")
                    nc.vector.tensor_scalar_mul(acc[:], rsum[:, 0:MT], oh_sb[:, 0:1])
                    for j in range(1, B):
                        nc.vector.scalar_tensor_tensor(
                            acc[:], rsum[:, j * MT:(j + 1) * MT], oh_sb[:, j:j + 1],
                            acc[:], ALU.mult, ALU.add)
                    d = bpool.tile([P, MT], F32, name=f"d{t}", tag="d")
                    nc.vector.scalar_tensor_tensor(
                        d[:], acc[:], g2f_sb[:, t:t + 1],
                        g2b2t_sb[:, t:t + 1].broadcast_to([P, MT]),
                        ALU.mult, ALU.add)
                    nc.vector.tensor_sub(d[:], d[:], mlp[t][:, 0:MT])
                    nc.vector.scalar_tensor_tensor(mlp[t][:, 0:MT], d[:], sel_sb[:, 0:1],
                                                   mlp[t][:, 0:MT], ALU.mult, ALU.add)

            _mark("p7b_blend")
            # ================= P8: final residual + out =================
            with tc.tile_pool(name="p8", bufs=2) as spool:
                for t in range(CT):
                    ot = spool.tile([P, TOK], F32, name=f"ot{t}", tag="ot")
                    nc.vector.tensor_add(ot[:], x1[t][:].bitcast(F32), mlp[t][:])
                    nc.sync.dma_start(out=outT_d.ap()[t * P:(t + 1) * P, :], in_=ot[:])
                    if debug:
                        nc.sync.dma_start(out=dbg_mlp_d.ap()[t * P:(t + 1) * P, :],
                                          in_=mlp[t][:])
            cm_mlp.__exit__(None, None, None)
            cm_x1.__exit__(None, None, None)

    _mark("p8_out")
    nc.compile()
    return nc


# ======================= host side =======================

def _cols(v):
    """[K*P]-vector -> [P, K] array (column k = slice k of the vector)."""
    return np.ascontiguousarray(np.asarray(v, np.float32).reshape(-1, P).T)


def _dr8(wT, ncols):
    """[C_in, ncols] transposed weight -> [P, C_in//P, ncols] fp8 DR slab."""
    ks = wT.shape[0] // P
    out = np.ascontiguousarray(
        (wT * WS).reshape(ks, P, ncols).transpose(1, 0, 2))
    return out.astype(ml_dtypes.float8_e4m3)


def _bf3(wT, ncols):
    """[C_in, ncols] transposed weight -> [P, C_in//P, ncols] bf16 slab."""
    ks = wT.shape[0] // P
    out = np.ascontiguousarray(wT.reshape(ks, P, ncols).transpose(1, 0, 2))
    return out.astype(ml_dtypes.bfloat16)


def prep_inputs(x, mask, relative_position_bias, norm1_w, norm1_b, qkv_w, q_bias,
                v_bias, proj_w, proj_b, gamma_1, gamma_2, norm2t_w, norm2t_b,
                t_fc1_w, t_fc1_b, t_fc2_w, t_fc2_b, norm2i_w, norm2i_b,
                i_fc1_w, i_fc1_b, i_fc2_w, i_fc2_b):
    f = lambda a: np.asarray(a, np.float32)
    x = f(x)
    rpb = f(relative_position_bias)
    mask = np.asarray(mask)

    qkv_w = f(qkv_w)
    wq = qkv_w[0:C] * SCALE
    wk = qkv_w[C:2 * C]
    wv = qkv_w[2 * C:3 * C]
    qb = f(q_bias) * SCALE
    vb = f(v_bias)
    # kind-major, rank-blocked: tile t = kind*8 + rank
    blocks = {0: [], 1: [], 2: []}
    bias_blocks = []
    for kind, w in ((0, wq), (1, wk), (2, wv)):
        for j in range(NCORE):
            blocks[kind].append(w[j * 2 * HD:(j + 1) * 2 * HD])
    for kind, bv in ((0, qb), (1, np.zeros(C, np.float32)), (2, vb)):
        bias_blocks.append(bv)
    # wqkv8[kind] = DR slab of (w_kind)^T: [P, CT, C]
    wqkv8 = np.stack([
        _dr8(np.ascontiguousarray(np.concatenate(blocks[k], axis=0).T), C)
        for k in range(3)], axis=0)
    qkvb = np.concatenate(bias_blocks)                              # [3C]

    erpbT_all = np.exp(rpb).transpose(0, 2, 1)                      # [H, k, q]
    xT = np.ascontiguousarray(x.reshape(B * N, C).T)                # [C, 8192]
    g1 = f(gamma_1)
    g2 = f(gamma_2)

    maskf = mask.astype(np.float32)                                 # [B, N]
    maskm = np.ascontiguousarray(maskf.reshape(B * KT, P).T)        # [P, B*KT]

    common = dict(
        wqkv8=wqkv8, qkvb=_cols(qkvb), maskm=maskm,
        wproj8=_dr8(np.ascontiguousarray(f(proj_w).T), C),
        projbg=_cols(g1 * f(proj_b)), g1q=_cols(g1 / (WS * CS)),
        n1w=_cols(norm1_w), n1b=_cols(norm1_b),
        n2iw=_cols(norm2i_w), n2ib=_cols(norm2i_b),
        n2tw=_cols(norm2t_w), n2tb=_cols(norm2t_b),
        wfc1i8=_dr8(np.ascontiguousarray(f(i_fc1_w).T), HID),
        bfc1i=_cols(i_fc1_b),
        wfc2i8=_dr8(np.ascontiguousarray(f(i_fc2_w).T), C),
        g2b2i=_cols(g2 * f(i_fc2_b)),
        g2q=_cols(g2 / WS), g2f=_cols(g2),
        g2b2t=_cols(g2 * f(t_fc2_b)),
    )

    w1tT = np.ascontiguousarray(f(t_fc1_w).T)    # [C, HID]
    w2tT = np.ascontiguousarray(f(t_fc2_w).T)    # [HID, C]
    b1t = f(t_fc1_b)

    in_maps = []
    for c in range(NCORE):
        im = dict(common)
        im["xT"] = np.ascontiguousarray(xT[:, c * TOK:(c + 1) * TOK])
        im["erpbT"] = np.ascontiguousarray(
            erpbT_all[c * HP:(c + 1) * HP]).astype(ml_dtypes.bfloat16)
        hs = slice(c * HSH, (c + 1) * HSH)
        im["wfc1t"] = _bf3(np.ascontiguousarray(w1tT[:, hs]), HSH)
        im["bfc1t"] = _cols(b1t[hs])
        im["wfc2t"] = _bf3(np.ascontiguousarray(w2tT[hs, :]), C)
        oh = np.zeros((P, B), np.float32)
        if c % 2 == 0:
            oh[:, c // 2] = 1.0
        im["oh"] = oh
        im["sel"] = np.full((P, 1), 1.0 if (c % 2 == 0) else 0.0, np.float32)
        in_maps.append(im)
    return in_maps


_NC_CACHE = {}


def get_nc(debug=False, apply_mask=False):
    key = (debug, apply_mask)
    if key not in _NC_CACHE:
        _NC_CACHE[key] = build_kernel(debug=debug, apply_mask=apply_mask)
    return _NC_CACHE[key]


def run(inputs, debug=False):
    am = not np.all(np.asarray(inputs["mask"]) == 1)
    nc = get_nc(debug=debug, apply_mask=am)
    in_maps = prep_inputs(**inputs)
    return run_bass_kernel_spmd(nc, in_maps, core_ids=list(range(NCORE)))


def kernel(**inputs):
    res = run(inputs, debug=False)
    outT = np.concatenate([res.results[c]["outT"] for c in range(NCORE)], axis=1)
    return np.ascontiguousarray(outT.T).reshape(B, N, C).astype(np.float32)
